# revision 31
# baseline (speedup 1.0000x reference)
"""AdaptiveGraphConv (Chebyshev K=3) Trainium2 kernel, 8-core data-parallel.

Math (per (batch,time) item, x_item [N,C]):
  M = D^-1/2 A D^-1/2  (normalized adjacency; L = I - M)
  Tx1 = L x = x - Mx;  Tx2 = 2 L Tx1 - x
  out = x W0 + Tx1 W1 + Tx2 W2 + b
      = x (W0-W2) + LX (W1+2W2) + (M LX) (-2W2) + b,  LX = x - Mx
Sharding: data-parallel over batch dim B=64 -> 8 batches/core. Laplacian,
weights, bias replicated. No collectives.

Per core: batches processed as 4 pairs; channel-major tensors are
[128=(2b x 64c), (n,t)]; node-major tensors are [n(<=128 x3), (t, 2b x 64c)].
On-chip compute in bf16 (f32 PSUM accumulation, f32 output).
"""
import os
import sys
import numpy as np

_TRN_REPO = "/opt/trn_rl_repo"
if _TRN_REPO not in sys.path:
    sys.path.insert(0, _TRN_REPO)


def _ensure_ntff_hook():
    """Make antenv.axon_hooks importable so NTFF profiling can register.

    The agent container's antenv stub lacks axon_hooks; trn_boot degrades
    silently without it. Writing the tiny registry module before concourse
    imports restores profiling. Harmless if already present.
    """
    src = (
        "_hook = None\n"
        "def set_axon_ntff_profile_hook(hook):\n"
        "    global _hook\n"
        "    _hook = hook\n"
        "def get_axon_ntff_profile_hook():\n"
        "    return _hook\n"
    )
    try:
        import antenv  # noqa
        base = os.path.dirname(antenv.__file__)
        path = os.path.join(base, "axon_hooks.py")
        if not os.path.exists(path):
            with open(path, "w") as f:
                f.write(src)
    except Exception:
        pass


_ensure_ntff_hook()

B, C, N, T, K = 64, 64, 325, 12, 3
NCORES = 8
B_LOC = B // NCORES          # 8 batches per core
NPAIRS = B_LOC // 2          # 4 pairs of batches
NT = N * T                   # 3900
CNT = [128, 128, 69]         # node chunk sizes (325 = 128+128+69)
NOFF = [0, 128, 256]

_cache = {}


def _build():
    import concourse.bass as bass  # noqa
    import concourse.bacc as bacc
    import concourse.mybir as mybir
    import concourse.tile as tile
    from concourse import masks
    from contextlib import ExitStack

    f32 = mybir.dt.float32
    bf16 = mybir.dt.bfloat16
    ALU = mybir.AluOpType
    AF = mybir.ActivationFunctionType

    nc = bacc.Bacc("TRN2", target_bir_lowering=False, debug=False,
                   num_devices=NCORES)
    x_ext = nc.dram_tensor("x", [B_LOC, C, N, T], f32, kind="ExternalInput")
    adj_ext = nc.dram_tensor("adj", [N, N], f32, kind="ExternalInput")
    w_ext = nc.dram_tensor("W", [K, C, C], f32, kind="ExternalInput")
    b_ext = nc.dram_tensor("b", [C], f32, kind="ExternalInput")
    out_ext = nc.dram_tensor("out", [B_LOC, C, N, T], f32,
                             kind="ExternalOutput")

    with tile.TileContext(nc) as tc, ExitStack() as ctx:
        const = ctx.enter_context(tc.tile_pool(name="const", bufs=1))
        psum1 = ctx.enter_context(
            tc.tile_pool(name="psum1", bufs=4, space="PSUM"))
        psum2 = ctx.enter_context(
            tc.tile_pool(name="psum2", bufs=2, space="PSUM"))

        idn = const.tile([128, 128], bf16)
        masks.make_identity(nc, idn[:])

        # ---- M = D^-1/2 A D^-1/2, three node-row tiles [cnt, 325] bf16 ----
        Af = [const.tile([128, N], f32, tag=f"a{i}", name=f"a{i}")
              for i in range(3)]
        for i in range(3):
            nc.sync.dma_start(Af[i][: CNT[i], :],
                              adj_ext.ap()[NOFF[i]: NOFF[i] + CNT[i], :])
        s_col = [const.tile([128, 1], f32, tag=f"s{i}", name=f"s{i}")
                 for i in range(3)]
        for i in range(3):
            d = const.tile([128, 1], f32, tag="dtmp")
            nc.vector.reduce_sum(d[: CNT[i], :], Af[i][: CNT[i], :],
                                 axis=mybir.AxisListType.X)
            nc.scalar.activation(d[: CNT[i], :], d[: CNT[i], :], AF.Sqrt)
            nc.vector.reciprocal(s_col[i][: CNT[i], :], d[: CNT[i], :])
        # s as a row vector [1, N] via tiny transposes (f32 path)
        idf = const.tile([128, 128], f32)
        masks.make_identity(nc, idf[:])
        ps_s = psum1.tile([1, N], f32, tag="ps512")
        for i in range(3):
            nc.tensor.matmul(ps_s[0:1, NOFF[i]: NOFF[i] + CNT[i]],
                             s_col[i][: CNT[i], 0:1], idf[: CNT[i], : CNT[i]],
                             is_transpose=True)
        s_row = const.tile([1, N], f32)
        nc.vector.tensor_copy(s_row[:], ps_s[:])
        # broadcast s_row to 128 partitions: ones[1,128].T @ s_row
        ones = const.tile([1, 128], f32)
        nc.vector.memset(ones[:], 1.0)
        ps_b = psum1.tile([128, N], f32, tag="ps512")
        nc.tensor.matmul(ps_b[:, :], ones[0:1, :], s_row[0:1, :])
        s_bc = const.tile([128, N], f32)
        nc.vector.tensor_copy(s_bc[:], ps_b[:])
        # M_i = (s_col * A * s_row) -> bf16
        M = [const.tile([128, N], bf16, tag=f"m{i}", name=f"m{i}")
             for i in range(3)]
        for i in range(3):
            nc.vector.tensor_mul(Af[i][: CNT[i], :], Af[i][: CNT[i], :],
                                 s_bc[: CNT[i], :])
            nc.vector.tensor_scalar_mul(M[i][: CNT[i], :], Af[i][: CNT[i], :],
                                        s_col[i][: CNT[i], 0:1])

        # ---- weight combos as block-diagonal [128,128] bf16 (2 copies) ----
        Wsb = const.tile([128, K, C], f32)
        for h in (0, 1):
            nc.sync.dma_start(Wsb[64 * h: 64 * h + 64, :, :],
                              w_ext.ap().rearrange("k c d -> c k d"))
        Wa = const.tile([128, 128], bf16)
        Wb = const.tile([128, 128], bf16)
        Wc = const.tile([128, 128], bf16)
        for wt in (Wa, Wb, Wc):
            nc.gpsimd.memset(wt[:], 0.0)
        for h in (0, 1):
            r = slice(64 * h, 64 * h + 64)
            # Wa = W0 - W2
            nc.vector.tensor_sub(Wa[r, r], Wsb[r, 0, :], Wsb[r, 2, :])
            # Wb = 2*W2 + W1
            nc.vector.scalar_tensor_tensor(Wb[r, r], Wsb[r, 2, :], 2.0,
                                           Wsb[r, 1, :], ALU.mult, ALU.add)
            # Wc = -2*W2
            nc.vector.tensor_scalar_mul(Wc[r, r], Wsb[r, 2, :], -2.0)

        bias = const.tile([128, 1], f32)
        for h in (0, 1):
            nc.sync.dma_start(bias[64 * h: 64 * h + 64, :], b_ext.ap())

        # ---- main loop over batch pairs ----
        xs_pool = ctx.enter_context(tc.tile_pool(name="xs", bufs=2))
        nm_pool = ctx.enter_context(tc.tile_pool(name="nm", bufs=2))
        cm_pool = ctx.enter_context(tc.tile_pool(name="cm", bufs=2))
        out_pool = ctx.enter_context(tc.tile_pool(name="outp", bufs=2))

        xf_tiles = {}

        def emit_loads(p):
            Xf = xs_pool.tile([128, N, T], f32, tag="xf", name="xf")
            Xs = xs_pool.tile([128, N, T], bf16, tag="xsb", name="xsb")
            for i in range(3):
                nsl = slice(NOFF[i], NOFF[i] + CNT[i])
                for h in (0, 1):
                    nc.sync.dma_start(Xf[64 * h: 64 * h + 64, nsl, :],
                                      x_ext.ap()[2 * p + h, :, nsl, :])
                nc.vector.tensor_copy(Xs[:, nsl, :], Xf[:, nsl, :])
            xf_tiles[p] = Xs

        emit_loads(0)
        for p in range(NPAIRS):
            Xs = xf_tiles.pop(p)
            Xs_flat = Xs[:].rearrange("p n t -> p (n t)")

            # node-major X: 3 tiles [n<=128, (t, 2b*c)]
            XN = [nm_pool.tile([128, T, 128], bf16, tag=f"xn{i}",
                               name=f"xn{i}") for i in range(3)]
            for i in range(3):
                for tg in range(2):
                    ps = psum1.tile([128, 6, 128], bf16, tag="ps512")
                    for tt in range(6):
                        t = tg * 6 + tt
                        nc.tensor.matmul(
                            ps[: CNT[i], tt, :],
                            Xs[:, NOFF[i]: NOFF[i] + CNT[i], t],
                            idn[:], is_transpose=True)
                    nc.scalar.activation(
                        XN[i][: CNT[i], 6 * tg: 6 * tg + 6, :],
                        ps[: CNT[i], :, :], AF.Copy)

            XNf = [XN[i][:].rearrange("p t b -> p (t b)") for i in range(3)]

            if p + 1 < NPAIRS:
                emit_loads(p + 1)

            # app1: MX = M @ X (node-major), fused evict LX = X - MX
            LXN = [nm_pool.tile([128, T, 128], bf16, tag=f"lxn{i}",
                                name=f"lxn{i}") for i in range(3)]
            LXNf = [LXN[i][:].rearrange("p t b -> p (t b)") for i in range(3)]
            for i in range(3):
                for fc in range(3):
                    fs = slice(512 * fc, 512 * fc + 512)
                    ps = psum1.tile([128, 512], f32, tag="ps512")
                    for j in range(3):
                        nc.tensor.matmul(
                            ps[: CNT[i], :],
                            M[j][: CNT[j], NOFF[i]: NOFF[i] + CNT[i]],
                            XNf[j][: CNT[j], fs],
                            start=(j == 0), stop=(j == 2))
                    nc.vector.tensor_tensor(LXNf[i][: CNT[i], fs],
                                            XNf[i][: CNT[i], fs],
                                            ps[: CNT[i], :], ALU.subtract)

            # app2: MLX = M @ LX via lhsT-form: out[(2b,c), n'] per t lands
            # channel-major directly (no back-transposes). Evict t-major
            # (contiguous), then one strided-read repack pass to n-major.
            MLXtm = cm_pool.tile([128, T, N], bf16, tag="mlxtm")
            for t in range(T):
                ps = psum2.tile([128, N], f32, tag="psb2")
                for j in range(3):
                    nc.tensor.matmul(
                        ps[:, :],
                        LXN[j][: CNT[j], t, :],
                        M[j][: CNT[j], :],
                        start=(j == 0), stop=(j == 2))
                nc.scalar.activation(MLXtm[:, t, :], ps[:, :], AF.Copy)
            MLXt = cm_pool.tile([128, N, T], bf16, tag="mlxt")
            nc.scalar.activation(MLXt[:],
                                 MLXtm[:].rearrange("p t n -> p n t"),
                                 AF.Copy)

            # LX back to channel-major [128=(2b,c), n, t] via PE transposes;
            # psum one n-window per 6 t; contiguous writes, strided psum reads
            LXt = cm_pool.tile([128, N, T], bf16, tag="lxt")
            for i in range(3):
                for tg in range(2):
                    ps = psum2.tile([128, 6, 128], bf16, tag="pst")
                    for tt in range(6):
                        t = 6 * tg + tt
                        nc.tensor.matmul(
                            ps[:, tt, : CNT[i]],
                            LXN[i][: CNT[i], t, :],
                            idn[: CNT[i], : CNT[i]], is_transpose=True)
                    dview = LXt[:, NOFF[i]: NOFF[i] + CNT[i],
                                6 * tg: 6 * tg + 6]
                    pview = ps[:, :, : CNT[i]].rearrange("p t n -> p n t")
                    nc.vector.tensor_copy(dview, pview)

            LXt_flat = LXt[:].rearrange("p n t -> p (n t)")
            MLXt_flat = MLXt[:].rearrange("p n t -> p (n t)")

            # W stage: out = Xs*Wa + LXt*Wb + MLXt*Wc + bias
            outsb = out_pool.tile([128, N, T], f32)
            out_flat = outsb[:].rearrange("p n t -> p (n t)")
            nchunk = (NT + 511) // 512
            for fc in range(nchunk):
                lo = 512 * fc
                hi = min(NT, lo + 512)
                fs = slice(lo, hi)
                ps = psum1.tile([128, 512], f32, tag="ps512")
                pw = ps[:, : hi - lo]
                nc.tensor.matmul(pw, Wa[:], Xs_flat[:, fs],
                                 start=True, stop=False)
                nc.tensor.matmul(pw, Wb[:], LXt_flat[:, fs],
                                 start=False, stop=False)
                nc.tensor.matmul(pw, Wc[:], MLXt_flat[:, fs],
                                 start=False, stop=True)
                nc.scalar.activation(out_flat[:, fs], pw, AF.Identity,
                                     bias=bias[:, 0:1])

            out_hbm = [out_ext.ap()[2 * p + h].rearrange("c n t -> c (n t)")
                       for h in (0, 1)]
            for h in (0, 1):
                for lo2, hi2 in ((0, 2048), (2048, NT)):
                    nc.sync.dma_start(out_hbm[h][:, lo2:hi2],
                                      out_flat[64 * h: 64 * h + 64, lo2:hi2])

    nc.compile()
    return nc


def _get_nc():
    if "nc" not in _cache:
        _cache["nc"] = _build()
    return _cache["nc"]


last_exec_time_ns = None
last_results = None


def kernel(x, adj, W, b):
    from concourse.bass_utils import run_bass_kernel_spmd

    global last_exec_time_ns, last_results
    nc = _get_nc()
    x = np.ascontiguousarray(x, dtype=np.float32)
    adj = np.ascontiguousarray(adj, dtype=np.float32)
    W = np.ascontiguousarray(W, dtype=np.float32)
    b = np.ascontiguousarray(b, dtype=np.float32)
    in_maps = [
        {"x": x[i * B_LOC: (i + 1) * B_LOC], "adj": adj, "W": W, "b": b}
        for i in range(NCORES)
    ]
    trace = bool(os.environ.get("KERNEL_TRACE"))
    res = run_bass_kernel_spmd(nc, in_maps, list(range(NCORES)), trace=trace)
    last_exec_time_ns = res.exec_time_ns
    last_results = res
    out = np.concatenate([res.results[i]["out"] for i in range(NCORES)],
                         axis=0)
    return out


# revision 32
# speedup vs baseline: 1.0685x; 1.0685x over previous
"""AdaptiveGraphConv (Chebyshev K=3) Trainium2 kernel, 8-core data-parallel.

Math (per (batch,time) item, x_item [N,C]):
  M = D^-1/2 A D^-1/2  (normalized adjacency; L = I - M)
  Tx1 = L x = x - Mx;  Tx2 = 2 L Tx1 - x
  out = x W0 + Tx1 W1 + Tx2 W2 + b
      = x (W0-W2) + LX (W1+2W2) + (M LX) (-2W2) + b,  LX = x - Mx
Sharding: data-parallel over batch dim B=64 -> 8 batches/core. Laplacian,
weights, bias replicated. No collectives.

Per core: batches processed as 4 pairs; channel-major tensors are
[128=(2b x 64c), (n,t)]; node-major tensors are [n(<=128 x3), (t, 2b x 64c)].
On-chip compute in bf16 (f32 PSUM accumulation, f32 output).
"""
import os
import sys
import numpy as np

_TRN_REPO = "/opt/trn_rl_repo"
if _TRN_REPO not in sys.path:
    sys.path.insert(0, _TRN_REPO)


def _ensure_ntff_hook():
    """Make antenv.axon_hooks importable so NTFF profiling can register.

    The agent container's antenv stub lacks axon_hooks; trn_boot degrades
    silently without it. Writing the tiny registry module before concourse
    imports restores profiling. Harmless if already present.
    """
    src = (
        "_hook = None\n"
        "def set_axon_ntff_profile_hook(hook):\n"
        "    global _hook\n"
        "    _hook = hook\n"
        "def get_axon_ntff_profile_hook():\n"
        "    return _hook\n"
    )
    try:
        import antenv  # noqa
        base = os.path.dirname(antenv.__file__)
        path = os.path.join(base, "axon_hooks.py")
        if not os.path.exists(path):
            with open(path, "w") as f:
                f.write(src)
    except Exception:
        pass


_ensure_ntff_hook()

B, C, N, T, K = 64, 64, 325, 12, 3
NCORES = 8
B_LOC = B // NCORES          # 8 batches per core
NPAIRS = B_LOC // 2          # 4 pairs of batches
NT = N * T                   # 3900
CNT = [128, 128, 69]         # node chunk sizes (325 = 128+128+69)
NOFF = [0, 128, 256]

_cache = {}


def _build():
    import concourse.bass as bass  # noqa
    import concourse.bacc as bacc
    import concourse.mybir as mybir
    import concourse.tile as tile
    from concourse import masks
    from contextlib import ExitStack

    f32 = mybir.dt.float32
    bf16 = mybir.dt.bfloat16
    ALU = mybir.AluOpType
    AF = mybir.ActivationFunctionType

    nc = bacc.Bacc("TRN2", target_bir_lowering=False, debug=False,
                   num_devices=NCORES)
    x_ext = nc.dram_tensor("x", [B_LOC, C, N, T], f32, kind="ExternalInput")
    adj_ext = nc.dram_tensor("adj", [N, N], f32, kind="ExternalInput")
    w_ext = nc.dram_tensor("W", [K, C, C], f32, kind="ExternalInput")
    b_ext = nc.dram_tensor("b", [C], f32, kind="ExternalInput")
    out_ext = nc.dram_tensor("out", [B_LOC, C, N, T], f32,
                             kind="ExternalOutput")

    with tile.TileContext(nc) as tc, ExitStack() as ctx:
        const = ctx.enter_context(tc.tile_pool(name="const", bufs=1))
        psum1 = ctx.enter_context(
            tc.tile_pool(name="psum1", bufs=5, space="PSUM"))
        psum2 = ctx.enter_context(
            tc.tile_pool(name="psum2", bufs=3, space="PSUM"))

        idn = const.tile([128, 128], bf16)
        masks.make_identity(nc, idn[:])

        # ---- M = D^-1/2 A D^-1/2, three node-row tiles [cnt, 325] bf16 ----
        Af = [const.tile([128, N], f32, tag=f"a{i}", name=f"a{i}")
              for i in range(3)]
        for i in range(3):
            nc.sync.dma_start(Af[i][: CNT[i], :],
                              adj_ext.ap()[NOFF[i]: NOFF[i] + CNT[i], :])
        s_col = [const.tile([128, 1], f32, tag=f"s{i}", name=f"s{i}")
                 for i in range(3)]
        for i in range(3):
            d = const.tile([128, 1], f32, tag="dtmp")
            nc.vector.reduce_sum(d[: CNT[i], :], Af[i][: CNT[i], :],
                                 axis=mybir.AxisListType.X)
            nc.scalar.activation(d[: CNT[i], :], d[: CNT[i], :], AF.Sqrt)
            nc.vector.reciprocal(s_col[i][: CNT[i], :], d[: CNT[i], :])
        # s as a row vector [1, N] via tiny transposes (f32 path)
        idf = const.tile([128, 128], f32)
        masks.make_identity(nc, idf[:])
        ps_s = psum1.tile([1, N], f32, tag="ps512")
        for i in range(3):
            nc.tensor.matmul(ps_s[0:1, NOFF[i]: NOFF[i] + CNT[i]],
                             s_col[i][: CNT[i], 0:1], idf[: CNT[i], : CNT[i]],
                             is_transpose=True)
        s_row = const.tile([1, N], f32)
        nc.vector.tensor_copy(s_row[:], ps_s[:])
        # broadcast s_row to 128 partitions: ones[1,128].T @ s_row
        ones = const.tile([1, 128], f32)
        nc.vector.memset(ones[:], 1.0)
        ps_b = psum1.tile([128, N], f32, tag="ps512")
        nc.tensor.matmul(ps_b[:, :], ones[0:1, :], s_row[0:1, :])
        s_bc = const.tile([128, N], f32)
        nc.vector.tensor_copy(s_bc[:], ps_b[:])
        # M_i = (s_col * A * s_row) -> bf16
        M = [const.tile([128, N], bf16, tag=f"m{i}", name=f"m{i}")
             for i in range(3)]
        for i in range(3):
            nc.vector.tensor_mul(Af[i][: CNT[i], :], Af[i][: CNT[i], :],
                                 s_bc[: CNT[i], :])
            nc.vector.tensor_scalar_mul(M[i][: CNT[i], :], Af[i][: CNT[i], :],
                                        s_col[i][: CNT[i], 0:1])

        # ---- weight combos as block-diagonal [128,128] bf16 (2 copies) ----
        Wsb = const.tile([128, K, C], f32)
        for h in (0, 1):
            nc.sync.dma_start(Wsb[64 * h: 64 * h + 64, :, :],
                              w_ext.ap().rearrange("k c d -> c k d"))
        Wa = const.tile([128, 128], bf16)
        Wb = const.tile([128, 128], bf16)
        Wc = const.tile([128, 128], bf16)
        for wt in (Wa, Wb, Wc):
            nc.gpsimd.memset(wt[:], 0.0)
        for h in (0, 1):
            r = slice(64 * h, 64 * h + 64)
            # Wa = W0 - W2
            nc.vector.tensor_sub(Wa[r, r], Wsb[r, 0, :], Wsb[r, 2, :])
            # Wb = 2*W2 + W1
            nc.vector.scalar_tensor_tensor(Wb[r, r], Wsb[r, 2, :], 2.0,
                                           Wsb[r, 1, :], ALU.mult, ALU.add)
            # Wc = -2*W2
            nc.vector.tensor_scalar_mul(Wc[r, r], Wsb[r, 2, :], -2.0)

        bias = const.tile([128, 1], f32)
        for h in (0, 1):
            nc.sync.dma_start(bias[64 * h: 64 * h + 64, :], b_ext.ap())

        # ---- main loop over batch pairs ----
        xs_pool = ctx.enter_context(tc.tile_pool(name="xs", bufs=2))
        nm_pool = ctx.enter_context(tc.tile_pool(name="nm", bufs=2))
        cm_pool = ctx.enter_context(tc.tile_pool(name="cm", bufs=2))
        out_pool = ctx.enter_context(tc.tile_pool(name="outp", bufs=2))

        xf_tiles = {}

        def emit_loads(p):
            Xf = xs_pool.tile([128, N, T], f32, tag="xf", name="xf")
            Xs = xs_pool.tile([128, N, T], bf16, tag="xsb", name="xsb")
            for i in range(3):
                nsl = slice(NOFF[i], NOFF[i] + CNT[i])
                for h in (0, 1):
                    nc.sync.dma_start(Xf[64 * h: 64 * h + 64, nsl, :],
                                      x_ext.ap()[2 * p + h, :, nsl, :])
                nc.vector.tensor_copy(Xs[:, nsl, :], Xf[:, nsl, :])
            xf_tiles[p] = Xs

        emit_loads(0)
        for p in range(NPAIRS):
            Xs = xf_tiles.pop(p)
            Xs_flat = Xs[:].rearrange("p n t -> p (n t)")

            # node-major X: 3 tiles [n<=128, (t, 2b*c)]
            XN = [nm_pool.tile([128, T, 128], bf16, tag=f"xn{i}",
                               name=f"xn{i}") for i in range(3)]
            for i in range(3):
                for tg in range(2):
                    ps = psum1.tile([128, 6, 128], bf16, tag="ps512")
                    for tt in range(6):
                        t = tg * 6 + tt
                        nc.tensor.matmul(
                            ps[: CNT[i], tt, :],
                            Xs[:, NOFF[i]: NOFF[i] + CNT[i], t],
                            idn[:], is_transpose=True)
                    nc.scalar.activation(
                        XN[i][: CNT[i], 6 * tg: 6 * tg + 6, :],
                        ps[: CNT[i], :, :], AF.Copy)

            XNf = [XN[i][:].rearrange("p t b -> p (t b)") for i in range(3)]

            if p + 1 < NPAIRS:
                emit_loads(p + 1)

            # app1: MX = M @ X (node-major), fused evict LX = X - MX
            LXN = [nm_pool.tile([128, T, 128], bf16, tag=f"lxn{i}",
                                name=f"lxn{i}") for i in range(3)]
            LXNf = [LXN[i][:].rearrange("p t b -> p (t b)") for i in range(3)]
            for i in range(3):
                for fc in range(3):
                    fs = slice(512 * fc, 512 * fc + 512)
                    ps = psum1.tile([128, 512], f32, tag="ps512")
                    for j in range(3):
                        nc.tensor.matmul(
                            ps[: CNT[i], :],
                            M[j][: CNT[j], NOFF[i]: NOFF[i] + CNT[i]],
                            XNf[j][: CNT[j], fs],
                            start=(j == 0), stop=(j == 2))
                    nc.vector.tensor_tensor(LXNf[i][: CNT[i], fs],
                                            XNf[i][: CNT[i], fs],
                                            ps[: CNT[i], :], ALU.subtract)

            # app2: MLX = M @ LX (node-major), plain evict
            MLXN = [nm_pool.tile([128, T, 128], bf16, tag=f"mlxn{i}",
                                 name=f"mlxn{i}") for i in range(3)]
            MLXNf = [MLXN[i][:].rearrange("p t b -> p (t b)")
                     for i in range(3)]
            for i in range(3):
                for fc in range(3):
                    fs = slice(512 * fc, 512 * fc + 512)
                    ps = psum1.tile([128, 512], f32, tag="ps512")
                    for j in range(3):
                        nc.tensor.matmul(
                            ps[: CNT[i], :],
                            M[j][: CNT[j], NOFF[i]: NOFF[i] + CNT[i]],
                            LXNf[j][: CNT[j], fs],
                            start=(j == 0), stop=(j == 2))
                    nc.scalar.activation(MLXNf[i][: CNT[i], fs],
                                         ps[: CNT[i], :], AF.Copy)

            # back to channel-major: LXt, MLXt [128=(2b,c), n, t].
            # psum collects one n-window per 6 t; eviction writes a
            # contiguous [cnt, 6] block (strided reads on the psum side).
            LXt = cm_pool.tile([128, N, T], bf16, tag="lxt")
            MLXt = cm_pool.tile([128, N, T], bf16, tag="mlxt")
            for src, dst, eng in ((LXN, LXt, "v"), (MLXN, MLXt, "s")):
                for i in range(3):
                    for tg in range(2):
                        ps = psum2.tile([128, 6, 128], bf16, tag="pst")
                        for tt in range(6):
                            t = 6 * tg + tt
                            nc.tensor.matmul(
                                ps[:, tt, : CNT[i]],
                                src[i][: CNT[i], t, :],
                                idn[: CNT[i], : CNT[i]], is_transpose=True)
                        dview = dst[:, NOFF[i]: NOFF[i] + CNT[i],
                                    6 * tg: 6 * tg + 6]
                        pview = ps[:, :, : CNT[i]].rearrange("p t n -> p n t")
                        if eng == "v":
                            nc.vector.tensor_copy(dview, pview)
                        else:
                            nc.scalar.activation(dview, pview, AF.Copy)

            LXt_flat = LXt[:].rearrange("p n t -> p (n t)")
            MLXt_flat = MLXt[:].rearrange("p n t -> p (n t)")

            # W stage: out = Xs*Wa + LXt*Wb + MLXt*Wc + bias
            outsb = out_pool.tile([128, N, T], f32)
            out_flat = outsb[:].rearrange("p n t -> p (n t)")
            nchunk = (NT + 511) // 512
            for fc in range(nchunk):
                lo = 512 * fc
                hi = min(NT, lo + 512)
                fs = slice(lo, hi)
                ps = psum1.tile([128, 512], f32, tag="ps512")
                pw = ps[:, : hi - lo]
                nc.tensor.matmul(pw, Wa[:], Xs_flat[:, fs],
                                 start=True, stop=False)
                nc.tensor.matmul(pw, Wb[:], LXt_flat[:, fs],
                                 start=False, stop=False)
                nc.tensor.matmul(pw, Wc[:], MLXt_flat[:, fs],
                                 start=False, stop=True)
                nc.scalar.activation(out_flat[:, fs], pw, AF.Identity,
                                     bias=bias[:, 0:1])

            out_hbm = [out_ext.ap()[2 * p + h].rearrange("c n t -> c (n t)")
                       for h in (0, 1)]
            for h in (0, 1):
                for lo2, hi2 in ((0, 2048), (2048, NT)):
                    nc.sync.dma_start(out_hbm[h][:, lo2:hi2],
                                      out_flat[64 * h: 64 * h + 64, lo2:hi2])

    nc.compile()
    return nc


def _get_nc():
    if "nc" not in _cache:
        _cache["nc"] = _build()
    return _cache["nc"]


last_exec_time_ns = None
last_results = None


def kernel(x, adj, W, b):
    from concourse.bass_utils import run_bass_kernel_spmd

    global last_exec_time_ns, last_results
    nc = _get_nc()
    x = np.ascontiguousarray(x, dtype=np.float32)
    adj = np.ascontiguousarray(adj, dtype=np.float32)
    W = np.ascontiguousarray(W, dtype=np.float32)
    b = np.ascontiguousarray(b, dtype=np.float32)
    in_maps = [
        {"x": x[i * B_LOC: (i + 1) * B_LOC], "adj": adj, "W": W, "b": b}
        for i in range(NCORES)
    ]
    trace = bool(os.environ.get("KERNEL_TRACE"))
    res = run_bass_kernel_spmd(nc, in_maps, list(range(NCORES)), trace=trace)
    last_exec_time_ns = res.exec_time_ns
    last_results = res
    out = np.concatenate([res.results[i]["out"] for i in range(NCORES)],
                         axis=0)
    return out
